# revision 17
# baseline (speedup 1.0000x reference)
"""TRN2 Bass kernel for nn_AttLayer (GNN TransformerConv message passing).

Strategy (8 NeuronCores, SPMD):
  - Nodes are sorted by in-degree (desc) and dealt band-by-band (1024 ranks
    = 128 rows x 8 cores) into (core, local-row) cells. Within each band a
    greedy chooses each node's core so that, for every destination node,
    its in-neighbors split evenly between two OVERLAPPING int16-index
    windows of the kv table: W0 = rows [0, 32768), W1 = rows [17408, 50176).
    Nodes in the overlap can serve either window, letting per-dst splits
    balance; this cuts gather slots from 1386 K-units (77% padding) to ~940
    (~20%).
  - Dense phase (per core): h = relu(x W_fc^T + b_fc); q/k/v/skip via PE.
    k|v packed per node into a 256-wide fp16 row, AllGathered so every core
    holds the full [50176, 256] table in HBM.
  - Edge phase (per core): dst nodes processed in groups of 128 (SBUF
    partitions). Group g has K1[g] window-0 slots and K2[g] window-1 slots
    (max in-degree-per-window over the group's rank band). Gathers are
    merged per PAIR of groups (amortizes SWDGE fixed cost): one dma_gather
    per (pair, window) on round-robin queues. Scores are per-slot fused
    multiply-reduce on DVE; softmax skips segment-max (scores bounded);
    normalization is folded into the edge weights (exv2 = ex*rden*valid)
    and the skip term seeds the fp16 aggregation chain, so there is no
    separate normalize/add pass.
  - Output rows are written in (core, lrow) order; the host inverse-permutes.

kernel(**inputs) takes the full unsharded inputs and returns the full
[50000, 128] float32 output.
"""

import numpy as np

import concourse.bacc as bacc
import concourse.bass as bass
import concourse.mybir as mybir
import concourse.tile as tile
from concourse.bass_utils import run_bass_kernel_spmd

F32 = mybir.dt.float32
F16 = mybir.dt.float16
I16 = mybir.dt.int16
AL = mybir.AluOpType
ACT = mybir.ActivationFunctionType

CFG = dict(N=50000, E=800000, D_IN=256, DH=128, DO=128, CORES=8)
SCALE = 1.0 / np.sqrt(128.0)
WIN = 32768  # int16 window size (rows)


def _wrap_idx16(grid):
    """[128, K] slot grid -> dma_gather idx tile [128, 8*K] int16."""
    K = grid.shape[1]
    stream = grid.T.reshape(-1)                    # [128*K], i = s*128+p
    w16 = stream.reshape(-1, 16).T                 # [16, 8*K]
    return np.tile(w16, (8, 1)).astype(np.int16)   # [128, 8*K]


def host_prep(inputs, cfg=CFG):
    N, E, CORES = cfg["N"], cfg["E"], cfg["CORES"]
    NL = N // CORES
    NLP = ((NL + 127) // 128) * 128
    NG = NLP // 128
    NROW = CORES * NLP
    B1 = NROW - WIN                      # window-1 base row

    x = np.ascontiguousarray(np.asarray(inputs["x"], np.float32))
    ei = np.asarray(inputs["edge_index"])
    src = ei[0].astype(np.int64)
    dst = ei[1].astype(np.int64)

    deg = np.bincount(dst, minlength=N)
    order = np.argsort(-deg, kind="stable")        # rank -> node id
    half = (deg + 1) // 2

    # out-edge CSR: src -> dsts
    osort = np.argsort(src, kind="stable")
    od = dst[osort]
    optr = np.zeros(N + 1, np.int64)
    optr[1:] = np.cumsum(np.bincount(src, minlength=N))

    # cell roles: 0 = only-W0, 2 = only-W1, 1 = overlap
    role = np.empty((NG, CORES), np.int8)
    for g in range(NG):
        for c in range(CORES):
            r0 = c * NLP + g * 128
            role[g, c] = 0 if r0 + 128 <= B1 else (2 if r0 >= WIN else 1)

    # greedy band-wise core assignment balancing per-dst window splits
    n0 = np.zeros(N, np.int32)
    n1 = np.zeros(N, np.int32)
    core_of = np.empty(N, np.int8)
    lrow_of = np.empty(N, np.int64)
    band = 128 * CORES

    def assign_band(g, nodes):
        cap = [128] * CORES
        lo_cores = [c for c in range(CORES) if role[g, c] == 0]
        hi_cores = [c for c in range(CORES) if role[g, c] == 2]
        mid_cores = [c for c in range(CORES) if role[g, c] == 1]
        for v in nodes:
            dl = od[optr[v]:optr[v + 1]]
            if len(dl):
                c_lo = int(np.sum(n0[dl] + 1 > half[dl]))
                c_hi = int(np.sum(n1[dl] + 1 > half[dl]))
            else:
                c_lo = c_hi = 0
            if c_lo < c_hi:
                pref = lo_cores + mid_cores + hi_cores
            elif c_hi < c_lo:
                pref = hi_cores + mid_cores + lo_cores
            else:
                p = int(np.sum(n0[dl] - n1[dl])) if len(dl) else 0
                pref = (hi_cores + mid_cores + lo_cores) if p > 0 else (
                    lo_cores + mid_cores + hi_cores)
            for c in pref:
                if cap[c] > 0:
                    break
            cap[c] -= 1
            core_of[v] = c
            lrow_of[v] = g * 128 + (128 - cap[c] - 1)
            r = c * NLP + lrow_of[v]
            if r < B1:
                n0[dl] += 1
            elif r >= WIN:
                n1[dl] += 1

    for g in range(NG):
        a, b = g * band, min((g + 1) * band, N)
        if a >= N:
            break
        assign_band(g, order[a:b])

    # refinement sweeps: re-assign each band against the full graph's
    # counts (first greedy pass only saw earlier bands)
    for _ in range(2):
        for g in range(NG):
            a, b = g * band, min((g + 1) * band, N)
            if a >= N:
                break
            nodes = order[a:b]
            for v in nodes:
                dl = od[optr[v]:optr[v + 1]]
                r = int(core_of[v]) * NLP + lrow_of[v]
                if r < B1:
                    n0[dl] -= 1
                elif r >= WIN:
                    n1[dl] -= 1
            assign_band(g, nodes)

    row_of = core_of.astype(np.int64) * NLP + lrow_of   # node -> table row

    # per-edge window assignment: forced outside the overlap; flexible
    # (overlap) edges top up window0 to each dst's half.
    erow = row_of[src]
    f_lo = erow < B1
    f_hi = erow >= WIN
    flex = ~f_lo & ~f_hi
    n0d = np.bincount(dst[f_lo], minlength=N)
    n1d = np.bincount(dst[f_hi], minlength=N)
    nfd = np.bincount(dst[flex], minlength=N)

    # per-group jointly-optimal caps (C1, C2) minimizing C1+C2, then the
    # per-dst flex allocation that realizes them
    grp_of = lrow_of // 128
    C1g = np.ones(NG, np.int64)
    C2g = np.ones(NG, np.int64)
    for g in range(NG):
        nodes = np.nonzero(grp_of == g)[0]
        if len(nodes) == 0:
            continue
        a0, a1, af = n0d[nodes], n1d[nodes], nfd[nodes]
        best = None
        for C1 in range(max(int(a0.max()), 1), int((a0 + af).max()) + 1):
            xmax = np.minimum(af, C1 - a0)
            C2 = max(int(np.max(a1 + af - xmax)), int(a1.max()), 1)
            if best is None or C1 + C2 < best[0] + best[1]:
                best = (C1, C2)
            if C1 >= best[0] + best[1]:
                break
        C1g[g], C2g[g] = best
    x_need = np.clip(n1d + nfd - C2g[grp_of], 0, nfd)  # flex edges sent to W0

    ewin = np.where(f_hi, 1, 0).astype(np.int64)
    # flexible edges: first x_need[dst] occurrences (in edge order) -> W0
    flex_idx = np.nonzero(flex)[0]
    fd = dst[flex_idx]
    forder = np.argsort(fd, kind="stable")
    fsorted = flex_idx[forder]
    fdst = fd[forder]
    pos_in_dst = np.arange(len(fsorted)) - np.concatenate(
        ([0], np.cumsum(np.bincount(fdst, minlength=N))))[fdst]
    ewin[fsorted] = np.where(pos_in_dst < x_need[fdst], 0, 1)

    d0 = np.bincount(dst[ewin == 0], minlength=N)
    d1 = deg - d0

    # per-group K per window (shared across cores) from the optimal caps
    K1s = [int(v) for v in C1g]
    K2s = [int(v) for v in C2g]
    for g in range(NG):
        nodes = np.nonzero(grp_of == g)[0]
        if len(nodes):
            assert int(d0[nodes].max()) <= K1s[g]
            assert int(d1[nodes].max()) <= K2s[g]

    # edges sorted by (dst-row, window): per-dst W0 run then W1 run
    edst_row = row_of[dst]
    eorder = np.lexsort((ewin, edst_row))
    e_row_s = erow[eorder]
    e_win_s = ewin[eorder]
    starts = np.zeros(NROW + 1, np.int64)
    cnt_by_row = np.bincount(edst_row, minlength=NROW)
    starts[1:] = np.cumsum(cnt_by_row)
    d0_by_row = np.zeros(NROW, np.int64)
    d0_by_row[row_of] = d0
    d1_by_row = np.zeros(NROW, np.int64)
    d1_by_row[row_of] = d1

    # adaptive chunk list for merged gathers: merge consecutive groups
    # while each window's slot sum stays under the tile cap
    CAP = 20
    pairs = []
    cur, s1, s2 = [], 0, 0
    for g in range(NG):
        if cur and (s1 + K1s[g] > CAP or s2 + K2s[g] > CAP or len(cur) >= 4):
            pairs.append(tuple(cur))
            cur, s1, s2 = [], 0, 0
        cur.append(g)
        s1 += K1s[g]
        s2 += K2s[g]
    if cur:
        pairs.append(tuple(cur))

    # dense-phase weights
    W_fcT = np.ascontiguousarray(np.asarray(inputs["W_fc"], np.float32).T)
    W_all = np.ascontiguousarray(np.concatenate(
        [np.asarray(inputs[w], np.float32).T
         for w in ("W_q", "W_k", "W_v", "W_skip")], axis=1))
    bias_all = np.ascontiguousarray(np.tile(np.concatenate(
        [np.asarray(inputs[b], np.float32)
         for b in ("b_q", "b_k", "b_v", "b_skip")])[None, :], (128, 1)))
    b_fc_col = np.ascontiguousarray(
        np.asarray(inputs["b_fc"], np.float32)[:, None])

    in_maps, nodes_per_core = [], []
    for c in range(CORES):
        sel = core_of == c
        nodes_c = np.nonzero(sel)[0]
        lrows_c = lrow_of[nodes_c]
        nodes_per_core.append((nodes_c, lrows_c))
        xT = np.zeros((cfg["D_IN"], NLP), np.float32)
        xT[:, lrows_c] = x[nodes_c].T

        idx_parts, valid_parts = [], []
        rows_c = c * NLP + np.arange(NLP)
        st_c = starts[rows_c]
        dd0 = d0_by_row[rows_c]
        dd1 = d1_by_row[rows_c]
        for pr in pairs:
            for w, Ks, base in ((0, K1s, 0), (1, K2s, B1)):
                grids = []
                for g in pr:
                    K = Ks[g]
                    p = np.arange(g * 128, (g + 1) * 128)
                    st = st_c[p] + (dd0[p] if w == 1 else 0)
                    d = dd1[p] if w == 1 else dd0[p]
                    offs = st[:, None] + np.arange(K)[None, :]
                    m = np.arange(K)[None, :] < d[:, None]
                    vals = np.where(m, e_row_s[np.minimum(offs, E - 1)] - base, 0)
                    grids.append(vals)
                grid = np.concatenate(grids, axis=1)
                assert grid.min() >= 0 and grid.max() < WIN
                idx_parts.append(_wrap_idx16(grid).ravel())
        for g in range(NG):
            p = np.arange(g * 128, (g + 1) * 128)
            K1, K2 = K1s[g], K2s[g]
            m0 = np.arange(K1)[None, :] < dd0[p][:, None]
            m1 = np.arange(K2)[None, :] < dd1[p][:, None]
            valid_parts.append(np.concatenate(
                [m0, m1], axis=1).astype(np.float32).ravel())

        in_maps.append({
            "xT": xT,
            "idx": np.ascontiguousarray(np.concatenate(idx_parts)),
            "valid": np.ascontiguousarray(np.concatenate(valid_parts)),
            "W_fcT": W_fcT, "W_all": W_all,
            "bias_all": bias_all, "b_fc": b_fc_col,
        })
    meta = dict(K1s=K1s, K2s=K2s, pairs=pairs, NL=NL, NLP=NLP, NG=NG, B1=B1)
    return in_maps, nodes_per_core, meta


def build_nc(meta, cfg=CFG):
    K1s, K2s, pairs = meta["K1s"], meta["K2s"], meta["pairs"]
    NLP, NG, B1 = meta["NLP"], meta["NG"], meta["B1"]
    CORES = cfg["CORES"]
    NIDX16 = 1024 * (sum(K1s) + sum(K2s))
    NSLOT = 128 * (sum(K1s) + sum(K2s))
    NROW = CORES * NLP

    nc = bacc.Bacc("TRN2", target_bir_lowering=False, debug=False,
                   num_devices=CORES, num_swdge_queues=4)
    xT = nc.dram_tensor("xT", [cfg["D_IN"], NLP], F32, kind="ExternalInput").ap()
    idx = nc.dram_tensor("idx", [NIDX16], I16, kind="ExternalInput").ap()
    valid = nc.dram_tensor("valid", [NSLOT], F32, kind="ExternalInput").ap()
    W_fcT = nc.dram_tensor("W_fcT", [cfg["D_IN"], 128], F32,
                           kind="ExternalInput").ap()
    W_all = nc.dram_tensor("W_all", [128, 512], F32, kind="ExternalInput").ap()
    bias_all = nc.dram_tensor("bias_all", [128, 512], F32,
                              kind="ExternalInput").ap()
    b_fc = nc.dram_tensor("b_fc", [128, 1], F32, kind="ExternalInput").ap()
    out = nc.dram_tensor("out", [NLP, 128], F32, kind="ExternalOutput").ap()

    qnum = [0]

    def next_q():
        q = qnum[0]
        qnum[0] = (q + 1) % 4
        return q

    with tile.TileContext(nc) as tc:
        with (
            tc.tile_pool(name="const", bufs=1) as cpool,
            tc.tile_pool(name="persist", bufs=1) as ppool,
            tc.tile_pool(name="work", bufs=3) as wpool,
            tc.tile_pool(name="edge", bufs=3) as epool,
            tc.tile_pool(name="gpool", bufs=6) as gpool,
            tc.tile_pool(name="accp", bufs=3) as apool,
            tc.tile_pool(name="psum", bufs=2, space="PSUM") as pspool,
            tc.tile_pool(name="psum2", bufs=2, space="PSUM") as pspool2,
            tc.tile_pool(name="dram", bufs=1, space="DRAM") as dpool,
        ):
            # ---- constants ----
            wfc_a = cpool.tile([128, 128], F32)
            wfc_b = cpool.tile([128, 128], F32)
            nc.sync.dma_start(out=wfc_a[:, :], in_=W_fcT[0:128, :])
            nc.sync.dma_start(out=wfc_b[:, :], in_=W_fcT[128:256, :])
            wall = cpool.tile([128, 512], F32)
            nc.sync.dma_start(out=wall[:, :], in_=W_all[:, :])
            ball = cpool.tile([128, 512], F32)
            nc.sync.dma_start(out=ball[:, :], in_=bias_all[:, :])
            bfc = cpool.tile([128, 1], F32)
            nc.sync.dma_start(out=bfc[:, :], in_=b_fc[:, :])

            # ---- persistent per-shard tensors ----
            q_sb = ppool.tile([128, NLP], F16)
            skip_sb = ppool.tile([128, NLP], F16)
            kv_shard = dpool.tile([NLP, 256], F16)
            kv_full = dpool.tile([NROW, 256], F16, addr_space="Shared")

            # ---- dense phase ----
            col = 0
            while col < NLP:
                ts = min(512, NLP - col)
                xa = wpool.tile([128, ts], F32, tag="xa")
                xb = wpool.tile([128, ts], F32, tag="xb")
                nc.sync.dma_start(out=xa[:, :], in_=xT[0:128, col:col + ts])
                nc.sync.dma_start(out=xb[:, :], in_=xT[128:256, col:col + ts])
                ph = pspool.tile([128, ts], F32, tag="ph")
                nc.tensor.matmul(ph[:, :], lhsT=wfc_a[:, :], rhs=xa[:, :],
                                 start=True, stop=False)
                nc.tensor.matmul(ph[:, :], lhsT=wfc_b[:, :], rhs=xb[:, :],
                                 start=False, stop=True)
                hT = wpool.tile([128, ts], F32, tag="hT")
                nc.scalar.activation(hT[:, :], ph[:, :], ACT.Relu,
                                     bias=bfc[:, :], scale=1.0)
                for sub in range(ts // 128):
                    nb = (col + sub * 128) // 128
                    po = pspool2.tile([128, 512], F32, tag="po")
                    nc.tensor.matmul(po[:, :],
                                     lhsT=hT[:, sub * 128:(sub + 1) * 128],
                                     rhs=wall[:, :], start=True, stop=True)
                    blk = slice(nb * 128, (nb + 1) * 128)
                    nc.vector.tensor_add(q_sb[:, blk], po[:, 0:128],
                                         ball[:, 0:128])
                    kv_t = wpool.tile([128, 256], F16, tag="kv_t")
                    nc.vector.tensor_add(kv_t[:, :], po[:, 128:384],
                                         ball[:, 128:384])
                    nc.sync.dma_start(out=kv_shard[blk, :], in_=kv_t[:, :])
                    nc.vector.tensor_add(skip_sb[:, blk], po[:, 384:512],
                                         ball[:, 384:512])
                col += ts

            # ---- allgather kv ----
            nc.gpsimd.collective_compute(
                "AllGather", AL.bypass,
                replica_groups=[list(range(CORES))],
                ins=[kv_shard[:, :]], outs=[kv_full[:, :]],
            )

            # ---- edge phase ----
            ibase = 0
            vbase = 0
            for pr in pairs:
                Kp1 = sum(K1s[g] for g in pr)
                Kp2 = sum(K2s[g] for g in pr)
                gts = []
                for Kw, base, tg in ((Kp1, 0, "w0"), (Kp2, B1, "w1")):
                    idxt = gpool.tile([128, 8 * Kw], I16, tag=f"idx{tg}")
                    nc.sync.dma_start(
                        out=idxt[:, :],
                        in_=idx[ibase:ibase + 1024 * Kw].rearrange(
                            "(p k) -> p k", k=8 * Kw))
                    ibase += 1024 * Kw
                    gt = gpool.tile([128, Kw * 256], F16, tag=f"gt{tg}")
                    nc.gpsimd.dma_gather(
                        gt[:, :].rearrange("p (k d) -> p k d", d=256),
                        kv_full[base:base + WIN, :],
                        idxt[:, :], num_idxs=128 * Kw,
                        num_idxs_reg=128 * Kw, elem_size=256, elem_step=256,
                        single_packet=False, queue_num=next_q())
                    gts.append(gt[:, :].rearrange("p (k d) -> p k d", d=256))

                off1 = 0
                off2 = 0
                for g in pr:
                    K1, K2 = K1s[g], K2s[g]
                    K = K1 + K2
                    blk = slice(g * 128, (g + 1) * 128)

                    def gslot(s):
                        if s < K1:
                            return gts[0], off1 + s
                        return gts[1], off2 + (s - K1)

                    validt = epool.tile([128, K], F32, tag="validt")
                    nc.sync.dma_start(
                        out=validt[:, :],
                        in_=valid[vbase:vbase + 128 * K].rearrange(
                            "(p k) -> p k", k=K))
                    vbase += 128 * K

                    sc = epool.tile([128, K], F32, tag="sc")
                    for s in range(K):
                        gv, si = gslot(s)
                        scr = epool.tile([128, 128], F16, tag="scr")
                        nc.vector.scalar_tensor_tensor(
                            out=scr[:, :], in0=gv[:, si, 0:128], scalar=SCALE,
                            in1=q_sb[:, blk], op0=AL.mult, op1=AL.mult,
                            accum_out=sc[:, s:s + 1])
                    ex = epool.tile([128, K], F32, tag="ex")
                    nc.scalar.activation(ex[:, :], sc[:, :], ACT.Exp)
                    exv = epool.tile([128, K], F32, tag="exv")
                    den = epool.tile([128, 1], F32, tag="den")
                    nc.vector.scalar_tensor_tensor(
                        out=exv[:, :], in0=ex[:, :], scalar=1.0,
                        in1=validt[:, :], op0=AL.mult, op1=AL.mult,
                        accum_out=den[:, :])
                    den2 = epool.tile([128, 1], F32, tag="den2")
                    nc.vector.tensor_scalar_add(den2[:, :], den[:, :], 1e-20)
                    rden = epool.tile([128, 1], F32, tag="rden")
                    nc.vector.reciprocal(rden[:, :], den2[:, :])
                    exv2 = epool.tile([128, K], F32, tag="exv2")
                    nc.vector.scalar_tensor_tensor(
                        out=exv2[:, :], in0=exv[:, :], scalar=rden[:, 0:1],
                        in1=validt[:, :], op0=AL.mult, op1=AL.mult)

                    # aggregation chain seeded with the skip term; the
                    # last link writes the f32 output tile directly.
                    prev = skip_sb[:, blk]
                    for s in range(K):
                        gv, si = gslot(s)
                        if s == K - 1:
                            acc = epool.tile([128, 128], F32, tag="outg")
                        else:
                            acc = apool.tile([128, 128], F16, tag="acc")
                        nc.vector.scalar_tensor_tensor(
                            out=acc[:, :], in0=gv[:, si, 128:256],
                            scalar=exv2[:, s:s + 1], in1=prev,
                            op0=AL.mult, op1=AL.add)
                        prev = acc[:, :]
                    nc.sync.dma_start(out=out[blk, :], in_=prev)
                    off1 += K1
                    off2 += K2

    nc.compile()
    return nc


def run(inputs, cfg=CFG, core_ids=None, trace=False, **run_kwargs):
    in_maps, nodes_per_core, meta = host_prep(inputs, cfg)
    nc = build_nc(meta, cfg)
    if core_ids is None:
        core_ids = list(range(cfg["CORES"]))
    res = run_bass_kernel_spmd(nc, in_maps, core_ids=core_ids, trace=trace,
                               **run_kwargs)
    out_full = np.zeros((cfg["N"], cfg["DO"]), np.float32)
    for c, (nodes_c, lrows_c) in enumerate(nodes_per_core):
        out_full[nodes_c] = res.results[c]["out"][lrows_c]
    return out_full, res


def kernel(**inputs) -> np.ndarray:
    out, _ = run(inputs)
    return out
